# revision 31
# baseline (speedup 1.0000x reference)
"""Trainium2 Bass kernel for nn_AlgebraicTriangulationNet.

For each (frame, joint) problem: build the 8x4 DLT matrix A, form M = A^T A
(4x4 symmetric), and find the eigenvector of the smallest eigenvalue as the
dominant eigenvector of adj(M), amplified by repeated matrix squaring (each
squaring doubles the log of the eigenvalue separation).  The output
v[:3]/v[3] is invariant to the scale/sign of v, so no vector normalization is
needed; trace renormalization every few squarings keeps fp32 in range.

Sharding: pure data-parallel over the frame dim B across 8 cores.

Layout per core: partition = frame mod 128, free = (frame_tile t)*17 + joint.
All state is fp32 planes inside [128, n_planes, N] SBUF tiles; every compute
op is an elementwise [128, T, 17]-shaped VectorE (or ScalarE square) op.
SBUF is managed with phase-scoped tile pools (raw/A -> M -> adj -> squarings
-> tournament) so everything fits at T = 64 tiles (N = 1088).
"""

from contextlib import ExitStack

import numpy as np

import concourse.bacc as bacc
import concourse.bass as bass
import concourse.mybir as mybir
import concourse.tile as tile

F32 = mybir.dt.float32
OP = mybir.AluOpType

NCORES = 8
B_FULL = 65536
V, J = 4, 17
P = 128

# packed symmetric 4x4 index
PAIRS = [(0, 0), (0, 1), (0, 2), (0, 3), (1, 1), (1, 2), (1, 3), (2, 2), (2, 3), (3, 3)]
IDX = {}
for _n, (_i, _j) in enumerate(PAIRS):
    IDX[(_i, _j)] = _n
    IDX[(_j, _i)] = _n

# 2x2 minors over column pairs (index order s0..s5 / c0..c5)
MINOR_COLS = [(0, 1), (0, 2), (0, 3), (1, 2), (1, 3), (2, 3)]

# adjugate upper entries: list of (sign, m-entry, minor) with first term +.
# c-minors (rows 2,3) feed entries (0,0),(0,1),(1,1); s-minors (rows 0,1)
# feed the rest.  c0 is never used.
ADJ_C = {
    (0, 0): [(+1, (1, 1), 5), (-1, (1, 2), 4), (+1, (1, 3), 3)],
    (0, 1): [(+1, (0, 2), 4), (-1, (0, 1), 5), (-1, (0, 3), 3)],
    (1, 1): [(+1, (0, 0), 5), (-1, (0, 2), 2), (+1, (0, 3), 1)],
}
ADJ_S = {
    (0, 2): [(+1, (1, 3), 5), (-1, (2, 3), 4), (+1, (3, 3), 3)],
    (0, 3): [(+1, (2, 2), 4), (-1, (1, 2), 5), (-1, (2, 3), 3)],
    (1, 2): [(+1, (2, 3), 2), (-1, (0, 3), 5), (-1, (3, 3), 1)],
    (1, 3): [(+1, (0, 2), 5), (-1, (2, 2), 2), (+1, (2, 3), 1)],
    (2, 2): [(+1, (0, 3), 4), (-1, (1, 3), 2), (+1, (3, 3), 0)],
    (2, 3): [(+1, (1, 2), 2), (-1, (0, 2), 4), (-1, (2, 3), 0)],
    (3, 3): [(+1, (0, 2), 3), (-1, (1, 2), 1), (+1, (2, 2), 0)],
}


def _ap(t, off, dims):
    """Free-dim sub-AP of an SBUF tile: keep partition dim, replace free dims."""
    a = t[:]
    return bass.AP(tensor=a.tensor, offset=a.offset + off,
                   ap=[list(a.ap[0])] + [list(d) for d in dims])


def _dram_ap(handle, off, dims):
    a = handle[:]
    return bass.AP(tensor=a.tensor, offset=a.offset + off,
                   ap=[list(d) for d in dims])


def build_nc(bf=B_FULL // NCORES, k_squarings=13, n_matvec=2, norm_every=5,
             dma_group=8, repeat=1):
    assert bf % P == 0
    T = bf // P
    N = T * J

    nc = bacc.Bacc(None, target_bir_lowering=False)

    pts_d = nc.dram_tensor("points", [bf, V, J, 2], F32, kind="ExternalInput")
    conf_d = nc.dram_tensor("confidences", [bf, V, J], F32, kind="ExternalInput")
    proj_d = nc.dram_tensor("proj_matricies", [bf, V, 3, 4], F32, kind="ExternalInput")
    out_d = nc.dram_tensor("out", [bf, J, 3], F32, kind="ExternalOutput")

    def pl(t, plane_idx):
        return _ap(t, plane_idx * N, [[J, T], [1, J]])

    def pair(t, k0, k1):
        # two planes of a tile as one [2, T, J] AP (any plane pair works)
        return _ap(t, k0 * N, [[(k1 - k0) * N, 2], [J, T], [1, J]])

    with tile.TileContext(nc) as tc:
        TT = nc.vector.tensor_tensor
        SQ = nc.scalar.square

        def one_pass():
            persist = ExitStack()
            ptmp = persist.enter_context(tc.tile_pool(name="tmp2", bufs=2))
            bb_st = ExitStack()
            pbb = bb_st.enter_context(tc.tile_pool(name="bb", bufs=2))
            # M = A^T A lives in the bb rotation (same shape as the B buffers)
            m_t = pbb.tile([P, 10, N], F32, tag="bb")

            # ---------------- phase 1: DMA in + M = A^T A ----------------
            with (
                tc.tile_pool(name="raw", bufs=1) as praw,
                tc.tile_pool(name="pa", bufs=1) as pa,
            ):
                pts_raw = praw.tile([P, T, 136], F32, tag="pts")
                conf_raw = praw.tile([P, T, 68], F32, tag="conf")
                proj_raw = praw.tile([P, T, 48], F32, tag="proj")
                # pts+proj first (first A-build ops need them), conf last;
                # alternate HWDGE (sync) / SWDGE (gpsimd) queue sets
                qs = [nc.sync, nc.gpsimd]
                qi = 0
                for dram, tile_, w in ((pts_d, pts_raw, 136),
                                       (proj_d, proj_raw, 48),
                                       (conf_d, conf_raw, 68)):
                    for g0 in range(0, T, dma_group):
                        g = min(dma_group, T - g0)
                        qs[qi % 2].dma_start(
                            out=tile_[:, g0:g0 + g, :],
                            in_=_dram_ap(dram, g0 * P * w,
                                         [[w, P], [w * P, g], [1, w]]))
                        qi += 1

                def pts_in(v, i):  # points[:, v, :, i] -> [T, J]
                    return _ap(pts_raw, v * 34 + i, [[136, T], [2, J]])

                def proj_bc(v, r, cc):  # proj[:, v, r, cc] broadcast over J
                    return _ap(proj_raw, v * 12 + r * 4 + cc, [[48, T], [0, J]])

                def conf_in(v):
                    return _ap(conf_raw, v * 17, [[68, T], [1, J]])

                for v in range(V):
                    a_t = pa.tile([P, 8, N], F32, tag="pa")
                    for i in range(2):
                        # all 4 columns in one op: stack over c (stride-1 in proj)
                        dst = _ap(a_t, i * 4 * N, [[N, 4], [J, T], [1, J]])
                        pts_b = _ap(pts_raw, v * 34 + i, [[0, 4], [136, T], [2, J]])
                        p2 = _ap(proj_raw, v * 12 + 8, [[1, 4], [48, T], [0, J]])
                        pr = _ap(proj_raw, v * 12 + i * 4, [[1, 4], [48, T], [0, J]])
                        cf = _ap(conf_raw, v * 17, [[0, 4], [68, T], [1, J]])
                        TT(dst, pts_b, p2, OP.mult)
                        TT(dst, dst, pr, OP.subtract)
                        TT(dst, dst, cf, OP.mult)
                    for (a, b) in PAIRS:
                        e = IDX[(a, b)]
                        t0 = ptmp.tile([P, 2, N], F32, tag="tmp2")
                        if a == b:
                            SQ(pair(t0, 0, 1), pair(a_t, a, 4 + a))
                        else:
                            TT(pair(t0, 0, 1), pair(a_t, a, 4 + a),
                               pair(a_t, b, 4 + b), OP.mult)
                        if v == 0:
                            TT(pl(m_t, e), pl(t0, 0), pl(t0, 1), OP.add)
                        else:
                            TT(pl(t0, 0), pl(t0, 0), pl(t0, 1), OP.add)
                            TT(pl(m_t, e), pl(m_t, e), pl(t0, 0), OP.add)

            # ---------------- phase 2: adjugate ----------------
            b_t = pbb.tile([P, 10, N], F32, tag="bb")

            def minor(dst, r0, r1, ca, cb):
                # m[r0,ca]*m[r1,cb] - m[r0,cb]*m[r1,ca]  (pair-product then sub)
                t0 = ptmp.tile([P, 2, N], F32, tag="tmp2")
                TT(pair(t0, 0, 1),
                   pair(m_t, IDX[(r0, ca)], IDX[(r0, cb)]),
                   pair(m_t, IDX[(r1, cb)], IDX[(r1, ca)]), OP.mult)
                TT(dst, pl(t0, 0), pl(t0, 1), OP.subtract)

            def adj_entries(spec, mn_t, mn_idx):
                for (i, j), terms in spec.items():
                    e = IDX[(i, j)]
                    t0 = ptmp.tile([P, 2, N], F32, tag="tmp2")
                    (s1, mij1, k1), (s2, mij2, k2), (s3, mij3, k3) = terms
                    assert s1 == +1 and s2 == -1
                    TT(pair(t0, 0, 1),
                       pair(m_t, IDX[mij1], IDX[mij2]),
                       pair(mn_t, mn_idx[k1], mn_idx[k2]), OP.mult)
                    TT(pl(t0, 0), pl(t0, 0), pl(t0, 1), OP.subtract)
                    TT(pl(t0, 1), pl(m_t, IDX[mij3]), pl(mn_t, mn_idx[k3]), OP.mult)
                    TT(pl(b_t, e), pl(t0, 0), pl(t0, 1),
                       OP.add if s3 == +1 else OP.subtract)

            with tc.tile_pool(name="mn", bufs=1) as pmn:
                # c-minors (rows 2,3): c1..c5 used
                mn_t = pmn.tile([P, 6, N], F32, tag="mn")
                cidx = {}
                for k, (ca, cb) in enumerate(MINOR_COLS):
                    if k == 0:
                        continue  # c0 unused
                    cidx[k] = len(cidx)
                    minor(pl(mn_t, cidx[k]), 2, 3, ca, cb)
                adj_entries(ADJ_C, mn_t, cidx)
                # s-minors (rows 0,1): all 6 used
                mn_t = pmn.tile([P, 6, N], F32, tag="mn")
                sidx = {k: k for k in range(6)}
                for k, (ca, cb) in enumerate(MINOR_COLS):
                    minor(pl(mn_t, sidx[k]), 0, 1, ca, cb)
                adj_entries(ADJ_S, mn_t, sidx)

            # ---------------- phase 3: normalize + squarings ----------------
            def normalize(bt):
                t0 = ptmp.tile([P, 2, N], F32, tag="tmp2")
                TT(pl(t0, 0), pl(bt, IDX[(0, 0)]), pl(bt, IDX[(1, 1)]), OP.add)
                TT(pl(t0, 1), pl(bt, IDX[(2, 2)]), pl(bt, IDX[(3, 3)]), OP.add)
                TT(pl(t0, 0), pl(t0, 0), pl(t0, 1), OP.add)
                nc.vector.reciprocal_approx_fast(out=pl(t0, 1), in_=pl(t0, 0))
                # scale all 10 planes in one stacked op (trinv broadcast)
                TT(_ap(bt, 0, [[N, 10], [J, T], [1, J]]),
                   _ap(bt, 0, [[N, 10], [J, T], [1, J]]),
                   _ap(t0, N, [[0, 10], [J, T], [1, J]]), OP.mult)

            # off-diag output groups with stride-1 packed destinations
            ODG = [[(0, 1), (0, 2), (0, 3)], [(1, 2), (1, 3)], [(2, 3)]]

            with (
                tc.tile_pool(name="tmp4", bufs=2) as ptmp4,
                tc.tile_pool(name="tmpg", bufs=1) as ptmpg,
            ):
                normalize(b_t)
                for it in range(k_squarings):
                    bn_t = pbb.tile([P, 10, N], F32, tag="bb")
                    # diagonal: out_ii = sum_k b[i,k]^2 (squares on ScalarE, paired)
                    for i in range(4):
                        t0 = ptmp4.tile([P, 4, N], F32, tag="tmp4")
                        SQ(pair(t0, 0, 1), pair(b_t, IDX[(i, 0)], IDX[(i, 1)]))
                        SQ(pair(t0, 2, 3), pair(b_t, IDX[(i, 2)], IDX[(i, 3)]))
                        TT(pl(t0, 0), pl(t0, 0), pl(t0, 1), OP.add)
                        TT(pl(t0, 2), pl(t0, 2), pl(t0, 3), OP.add)
                        TT(pl(bn_t, IDX[(i, i)]), pl(t0, 0), pl(t0, 2), OP.add)
                    # off-diagonal: pair-products, then adds stacked per group
                    for grp in ODG:
                        ng = len(grp)
                        tg = ptmpg.tile([P, 3, 4, N], F32, tag="tmpg")
                        for m, (i, j) in enumerate(grp):
                            TT(_ap(tg, (m * 4) * N, [[N, 2], [J, T], [1, J]]),
                               pair(b_t, IDX[(i, 0)], IDX[(i, 1)]),
                               pair(b_t, IDX[(0, j)], IDX[(1, j)]), OP.mult)
                            TT(_ap(tg, (m * 4 + 2) * N, [[N, 2], [J, T], [1, J]]),
                               pair(b_t, IDX[(i, 2)], IDX[(i, 3)]),
                               pair(b_t, IDX[(2, j)], IDX[(3, j)]), OP.mult)
                        # stacked adds over the group
                        def gview(off):
                            return _ap(tg, off * N, [[4 * N, ng], [J, T], [1, J]])
                        TT(gview(0), gview(0), gview(1), OP.add)
                        TT(gview(2), gview(2), gview(3), OP.add)
                        e0 = IDX[grp[0]]
                        TT(_ap(bn_t, e0 * N, [[N, ng], [J, T], [1, J]]),
                           gview(0), gview(2), OP.add)
                    b_t = bn_t
                    if (it + 1) % norm_every == 0 and it != k_squarings - 1:
                        normalize(b_t)

            # ---------------- phase 4: select dominant column + output ----------
            with (
                tc.tile_pool(name="w1", bufs=1) as pw1,
                tc.tile_pool(name="wfp", bufs=1) as pwf,
                tc.tile_pool(name="msk", bufs=1) as pmsk,
                tc.tile_pool(name="oi", bufs=1) as poi,
            ):
                w1 = pw1.tile([P, 12, N], F32, tag="w1")
                msk = pmsk.tile([P, 3, N], mybir.dt.uint8, tag="msk")

                def mpl(k):
                    return _ap(msk, k * N, [[J, T], [1, J]])

                def sel(out, cmpp, on_true, on_false):
                    nc.scalar.copy(out, on_false)
                    nc.vector.copy_predicated(out, cmpp, on_true)

                def round1(base, mki, c0, c1):
                    cmpp = mpl(mki)
                    TT(cmpp, pl(b_t, IDX[(c0, c0)]), pl(b_t, IDX[(c1, c1)]), OP.is_ge)
                    for r in range(4):
                        sel(pl(w1, base + r), cmpp,
                            pl(b_t, IDX[(r, c0)]), pl(b_t, IDX[(r, c1)]))
                    TT(pl(w1, base + 4), pl(b_t, IDX[(c0, c0)]),
                       pl(b_t, IDX[(c1, c1)]), OP.max)

                round1(0, 0, 0, 1)
                round1(6, 1, 2, 3)
                wf = pwf.tile([P, 6, N], F32, tag="wf")
                cmpp = mpl(2)
                TT(cmpp, pl(w1, 4), pl(w1, 10), OP.is_ge)
                for r in range(4):
                    sel(pl(wf, r), cmpp, pl(w1, r), pl(w1, 6 + r))

                # refine: w <- B^(n_matvec) w  (each matvec adds 2^k_squarings to
                # the suppression exponent of the power iteration)
                wcur, walt = (wf, 0), (w1, 6)  # (tile, base plane)
                for _ in range(n_matvec):
                    wt, wb = wcur
                    at, ab = walt
                    for i in range(4):
                        # scratch: w1 planes 4,5 and 10,11 (dead after round 2)
                        TT(pair(w1, 4, 5), pair(b_t, IDX[(i, 0)], IDX[(i, 1)]),
                           pair(wt, wb, wb + 1), OP.mult)
                        TT(pair(w1, 10, 11), pair(b_t, IDX[(i, 2)], IDX[(i, 3)]),
                           pair(wt, wb + 2, wb + 3), OP.mult)
                        TT(pl(w1, 4), pl(w1, 4), pl(w1, 5), OP.add)
                        TT(pl(w1, 10), pl(w1, 10), pl(w1, 11), OP.add)
                        TT(pl(at, ab + i), pl(w1, 4), pl(w1, 10), OP.add)
                    wcur, walt = walt, wcur

                wt, wb = wcur
                rec = pl(wf, 4)
                nc.vector.reciprocal_approx_accurate(out=rec, in_=pl(wt, wb + 3),
                                                     scratch=pl(w1, 11))
                # interleave out-muls and out-DMA by halves so the final DMA
                # tail overlaps the second half's muls
                oi_t = poi.tile([P, T, 51], F32, tag="oi")
                half = max(dma_group, T // 2)
                for h0 in range(0, T, half):
                    hh = min(half, T - h0)
                    for d in range(3):
                        TT(_ap(oi_t, h0 * 51 + d, [[51, hh], [3, J]]),
                           _ap(wt, (wb + d) * N + h0 * J, [[J, hh], [1, J]]),
                           _ap(wf, 4 * N + h0 * J, [[J, hh], [1, J]]), OP.mult)
                    for g0 in range(h0, h0 + hh, dma_group):
                        g = min(dma_group, h0 + hh - g0)
                        qs[qi % 2].dma_start(
                            out=_dram_ap(out_d, g0 * P * 51,
                                         [[51, P], [51 * P, g], [1, 51]]),
                            in_=oi_t[:, g0:g0 + g, :])
                        qi += 1

            bb_st.close()
            persist.close()

        for _rep in range(repeat):
            one_pass()

    nc.compile()
    return nc


_RUNNER = None


def _get_runner():
    global _RUNNER
    if _RUNNER is None:
        import jax
        from jax.experimental.shard_map import shard_map
        from jax.sharding import Mesh, PartitionSpec

        from concourse import bass2jax

        nc = build_nc()
        bass2jax.install_neuronx_cc_hook()

        partition_name = (nc.partition_id_tensor.name
                          if nc.partition_id_tensor else None)
        in_names, out_names, out_avals, out_shapes = [], [], [], []
        for alloc in nc.m.functions[0].allocations:
            if not isinstance(alloc, mybir.MemoryLocationSet):
                continue
            name = alloc.memorylocations[0].name
            if alloc.kind == "ExternalInput":
                if name != partition_name:
                    in_names.append(name)
            elif alloc.kind == "ExternalOutput":
                out_names.append(name)
                shape = tuple(alloc.tensor_shape)
                dt = mybir.dt.np(alloc.dtype)
                out_avals.append(jax.core.ShapedArray(shape, dt))
                out_shapes.append((shape, dt))
        n_params = len(in_names)
        all_names = in_names + out_names
        if partition_name is not None:
            all_names = all_names + [partition_name]
        all_names = tuple(all_names)
        donate = tuple(range(n_params, n_params + len(out_names)))

        def _body(*args):
            operands = list(args)
            if partition_name is not None:
                operands.append(bass2jax.partition_id_tensor())
            outs = bass2jax._bass_exec_p.bind(
                *operands,
                out_avals=tuple(out_avals),
                in_names=all_names,
                out_names=tuple(out_names),
                lowering_input_output_aliases=(),
                sim_require_finite=True,
                sim_require_nnan=True,
                nc=nc,
            )
            return tuple(outs)

        import jax.numpy as jnp

        devices = jax.devices()[:NCORES]
        mesh = Mesh(np.asarray(devices), ("core",))
        nio = n_params + len(out_names)
        sharded = jax.jit(
            shard_map(_body, mesh=mesh,
                      in_specs=(PartitionSpec("core"),) * nio,
                      out_specs=(PartitionSpec("core"),) * len(out_names),
                      check_rep=False),
            donate_argnums=donate, keep_unused=True)
        in_sharding = jax.sharding.NamedSharding(mesh, PartitionSpec("core"))
        # device-side zero maker: donated output buffers without host upload
        zmaker = jax.jit(
            lambda: tuple(jnp.zeros((NCORES * s[0], *s[1:]), d)
                          for (s, d) in out_shapes),
            out_shardings=(in_sharding,) * len(out_shapes))
        _RUNNER = (sharded, in_names, out_names, out_shapes, in_sharding, zmaker)
    return _RUNNER


def kernel(points, confidences, proj_matricies, batch=None, **_unused):
    sharded, in_names, out_names, out_shapes, in_sharding, zmaker = _get_runner()

    arrs = {
        "points": np.ascontiguousarray(np.asarray(points, dtype=np.float32)),
        "confidences": np.ascontiguousarray(np.asarray(confidences, np.float32)),
        "proj_matricies": np.ascontiguousarray(np.asarray(proj_matricies, np.float32)),
    }
    import jax
    ins = [jax.device_put(arrs[n], in_sharding) for n in in_names]
    out_arrs = sharded(*ins, *zmaker())
    out = np.asarray(out_arrs[out_names.index("out")])
    return out.astype(np.float32)



# revision 36
# speedup vs baseline: 1.6207x; 1.6207x over previous
"""Trainium2 Bass kernel for nn_AlgebraicTriangulationNet.

For each (frame, joint) problem: build the 8x4 DLT matrix A, form M = A^T A
(4x4 symmetric), and find the eigenvector of the smallest eigenvalue as the
dominant eigenvector of adj(M), amplified by repeated matrix squaring (each
squaring doubles the log of the eigenvalue separation).  The output
v[:3]/v[3] is invariant to the scale/sign of v, so no vector normalization is
needed; trace renormalization every few squarings keeps fp32 in range.

Sharding: pure data-parallel over the frame dim B across 8 cores.

Layout per core: partition = frame mod 128, free = (frame_tile t)*17 + joint.
All state is fp32 planes inside [128, n_planes, N] SBUF tiles; every compute
op is an elementwise [128, T, 17]-shaped VectorE (or ScalarE square) op.
SBUF is managed with phase-scoped tile pools (raw/A -> M -> adj -> squarings
-> tournament) so everything fits at T = 64 tiles (N = 1088).
"""

from contextlib import ExitStack

import numpy as np

import concourse.bacc as bacc
import concourse.bass as bass
import concourse.mybir as mybir
import concourse.tile as tile

F32 = mybir.dt.float32
OP = mybir.AluOpType

NCORES = 8
B_FULL = 65536
V, J = 4, 17
P = 128

# packed symmetric 4x4 index
PAIRS = [(0, 0), (0, 1), (0, 2), (0, 3), (1, 1), (1, 2), (1, 3), (2, 2), (2, 3), (3, 3)]
IDX = {}
for _n, (_i, _j) in enumerate(PAIRS):
    IDX[(_i, _j)] = _n
    IDX[(_j, _i)] = _n

# 2x2 minors over column pairs (index order s0..s5 / c0..c5)
MINOR_COLS = [(0, 1), (0, 2), (0, 3), (1, 2), (1, 3), (2, 3)]

# adjugate upper entries: list of (sign, m-entry, minor) with first term +.
# c-minors (rows 2,3) feed entries (0,0),(0,1),(1,1); s-minors (rows 0,1)
# feed the rest.  c0 is never used.
ADJ_C = {
    (0, 0): [(+1, (1, 1), 5), (-1, (1, 2), 4), (+1, (1, 3), 3)],
    (0, 1): [(+1, (0, 2), 4), (-1, (0, 1), 5), (-1, (0, 3), 3)],
    (1, 1): [(+1, (0, 0), 5), (-1, (0, 2), 2), (+1, (0, 3), 1)],
}
ADJ_S = {
    (0, 2): [(+1, (1, 3), 5), (-1, (2, 3), 4), (+1, (3, 3), 3)],
    (0, 3): [(+1, (2, 2), 4), (-1, (1, 2), 5), (-1, (2, 3), 3)],
    (1, 2): [(+1, (2, 3), 2), (-1, (0, 3), 5), (-1, (3, 3), 1)],
    (1, 3): [(+1, (0, 2), 5), (-1, (2, 2), 2), (+1, (2, 3), 1)],
    (2, 2): [(+1, (0, 3), 4), (-1, (1, 3), 2), (+1, (3, 3), 0)],
    (2, 3): [(+1, (1, 2), 2), (-1, (0, 2), 4), (-1, (2, 3), 0)],
    (3, 3): [(+1, (0, 2), 3), (-1, (1, 2), 1), (+1, (2, 2), 0)],
}


def _ap(t, off, dims):
    """Free-dim sub-AP of an SBUF tile: keep partition dim, replace free dims."""
    a = t[:]
    return bass.AP(tensor=a.tensor, offset=a.offset + off,
                   ap=[list(a.ap[0])] + [list(d) for d in dims])


def _dram_ap(handle, off, dims):
    a = handle[:]
    return bass.AP(tensor=a.tensor, offset=a.offset + off,
                   ap=[list(d) for d in dims])


def build_nc(bf=B_FULL // NCORES, k_squarings=7, m_2x2=8, norm_every=5,
             dma_group=8, repeat=1):
    assert bf % P == 0
    T = bf // P
    N = T * J

    nc = bacc.Bacc(None, target_bir_lowering=False)

    pts_d = nc.dram_tensor("points", [bf, V, J, 2], F32, kind="ExternalInput")
    conf_d = nc.dram_tensor("confidences", [bf, V, J], F32, kind="ExternalInput")
    proj_d = nc.dram_tensor("proj_matricies", [bf, V, 3, 4], F32, kind="ExternalInput")
    out_d = nc.dram_tensor("out", [bf, J, 3], F32, kind="ExternalOutput")

    def pl(t, plane_idx):
        return _ap(t, plane_idx * N, [[J, T], [1, J]])

    def pair(t, k0, k1):
        # two planes of a tile as one [2, T, J] AP (any plane pair works)
        return _ap(t, k0 * N, [[(k1 - k0) * N, 2], [J, T], [1, J]])

    with tile.TileContext(nc) as tc:
        TT = nc.vector.tensor_tensor
        SQ = nc.scalar.square

        def one_pass():
            persist = ExitStack()
            ptmp = persist.enter_context(tc.tile_pool(name="tmp2", bufs=2))
            bb_st = ExitStack()
            pbb = bb_st.enter_context(tc.tile_pool(name="bb", bufs=2))
            # M = A^T A lives in the bb rotation (same shape as the B buffers)
            m_t = pbb.tile([P, 10, N], F32, tag="bb")

            # ---------------- phase 1: DMA in + M = A^T A ----------------
            with (
                tc.tile_pool(name="raw", bufs=1) as praw,
                tc.tile_pool(name="pa", bufs=1) as pa,
            ):
                pts_raw = praw.tile([P, T, 136], F32, tag="pts")
                conf_raw = praw.tile([P, T, 68], F32, tag="conf")
                proj_raw = praw.tile([P, T, 48], F32, tag="proj")
                # pts+proj first (first A-build ops need them), conf last;
                # alternate HWDGE (sync) / SWDGE (gpsimd) queue sets
                qs = [nc.sync, nc.gpsimd]
                qi = 0
                for dram, tile_, w in ((pts_d, pts_raw, 136),
                                       (proj_d, proj_raw, 48),
                                       (conf_d, conf_raw, 68)):
                    for g0 in range(0, T, dma_group):
                        g = min(dma_group, T - g0)
                        qs[qi % 2].dma_start(
                            out=tile_[:, g0:g0 + g, :],
                            in_=_dram_ap(dram, g0 * P * w,
                                         [[w, P], [w * P, g], [1, w]]))
                        qi += 1

                def pts_in(v, i):  # points[:, v, :, i] -> [T, J]
                    return _ap(pts_raw, v * 34 + i, [[136, T], [2, J]])

                def proj_bc(v, r, cc):  # proj[:, v, r, cc] broadcast over J
                    return _ap(proj_raw, v * 12 + r * 4 + cc, [[48, T], [0, J]])

                def conf_in(v):
                    return _ap(conf_raw, v * 17, [[68, T], [1, J]])

                for v in range(V):
                    a_t = pa.tile([P, 8, N], F32, tag="pa")
                    for i in range(2):
                        # all 4 columns in one op: stack over c (stride-1 in proj)
                        dst = _ap(a_t, i * 4 * N, [[N, 4], [J, T], [1, J]])
                        pts_b = _ap(pts_raw, v * 34 + i, [[0, 4], [136, T], [2, J]])
                        p2 = _ap(proj_raw, v * 12 + 8, [[1, 4], [48, T], [0, J]])
                        pr = _ap(proj_raw, v * 12 + i * 4, [[1, 4], [48, T], [0, J]])
                        cf = _ap(conf_raw, v * 17, [[0, 4], [68, T], [1, J]])
                        TT(dst, pts_b, p2, OP.mult)
                        TT(dst, dst, pr, OP.subtract)
                        TT(dst, dst, cf, OP.mult)
                    for (a, b) in PAIRS:
                        e = IDX[(a, b)]
                        t0 = ptmp.tile([P, 2, N], F32, tag="tmp2")
                        if a == b:
                            SQ(pair(t0, 0, 1), pair(a_t, a, 4 + a))
                        else:
                            TT(pair(t0, 0, 1), pair(a_t, a, 4 + a),
                               pair(a_t, b, 4 + b), OP.mult)
                        if v == 0:
                            TT(pl(m_t, e), pl(t0, 0), pl(t0, 1), OP.add)
                        else:
                            TT(pl(t0, 0), pl(t0, 0), pl(t0, 1), OP.add)
                            TT(pl(m_t, e), pl(m_t, e), pl(t0, 0), OP.add)

            # ---------------- phase 2: adjugate ----------------
            b_t = pbb.tile([P, 10, N], F32, tag="bb")

            def minor(dst, r0, r1, ca, cb):
                # m[r0,ca]*m[r1,cb] - m[r0,cb]*m[r1,ca]  (pair-product then sub)
                t0 = ptmp.tile([P, 2, N], F32, tag="tmp2")
                TT(pair(t0, 0, 1),
                   pair(m_t, IDX[(r0, ca)], IDX[(r0, cb)]),
                   pair(m_t, IDX[(r1, cb)], IDX[(r1, ca)]), OP.mult)
                TT(dst, pl(t0, 0), pl(t0, 1), OP.subtract)

            def adj_entries(spec, mn_t, mn_idx):
                for (i, j), terms in spec.items():
                    e = IDX[(i, j)]
                    t0 = ptmp.tile([P, 2, N], F32, tag="tmp2")
                    (s1, mij1, k1), (s2, mij2, k2), (s3, mij3, k3) = terms
                    assert s1 == +1 and s2 == -1
                    TT(pair(t0, 0, 1),
                       pair(m_t, IDX[mij1], IDX[mij2]),
                       pair(mn_t, mn_idx[k1], mn_idx[k2]), OP.mult)
                    TT(pl(t0, 0), pl(t0, 0), pl(t0, 1), OP.subtract)
                    TT(pl(t0, 1), pl(m_t, IDX[mij3]), pl(mn_t, mn_idx[k3]), OP.mult)
                    TT(pl(b_t, e), pl(t0, 0), pl(t0, 1),
                       OP.add if s3 == +1 else OP.subtract)

            with tc.tile_pool(name="mn", bufs=1) as pmn:
                # c-minors (rows 2,3): c1..c5 used
                mn_t = pmn.tile([P, 6, N], F32, tag="mn")
                cidx = {}
                for k, (ca, cb) in enumerate(MINOR_COLS):
                    if k == 0:
                        continue  # c0 unused
                    cidx[k] = len(cidx)
                    minor(pl(mn_t, cidx[k]), 2, 3, ca, cb)
                adj_entries(ADJ_C, mn_t, cidx)
                # s-minors (rows 0,1): all 6 used
                mn_t = pmn.tile([P, 6, N], F32, tag="mn")
                sidx = {k: k for k in range(6)}
                for k, (ca, cb) in enumerate(MINOR_COLS):
                    minor(pl(mn_t, sidx[k]), 0, 1, ca, cb)
                adj_entries(ADJ_S, mn_t, sidx)

            # ---------------- phase 3: normalize + squarings ----------------
            def normalize(bt):
                t0 = ptmp.tile([P, 2, N], F32, tag="tmp2")
                TT(pl(t0, 0), pl(bt, IDX[(0, 0)]), pl(bt, IDX[(1, 1)]), OP.add)
                TT(pl(t0, 1), pl(bt, IDX[(2, 2)]), pl(bt, IDX[(3, 3)]), OP.add)
                TT(pl(t0, 0), pl(t0, 0), pl(t0, 1), OP.add)
                nc.vector.reciprocal_approx_fast(out=pl(t0, 1), in_=pl(t0, 0))
                # scale all 10 planes in one stacked op (trinv broadcast)
                TT(_ap(bt, 0, [[N, 10], [J, T], [1, J]]),
                   _ap(bt, 0, [[N, 10], [J, T], [1, J]]),
                   _ap(t0, N, [[0, 10], [J, T], [1, J]]), OP.mult)

            # off-diag output groups with stride-1 packed destinations
            ODG = [[(0, 1), (0, 2), (0, 3)], [(1, 2), (1, 3)], [(2, 3)]]

            with (
                tc.tile_pool(name="tmp4", bufs=2) as ptmp4,
                tc.tile_pool(name="tmpg", bufs=1) as ptmpg,
            ):
                normalize(b_t)
                for it in range(k_squarings):
                    bn_t = pbb.tile([P, 10, N], F32, tag="bb")
                    # diagonal: out_ii = sum_k b[i,k]^2 (squares on ScalarE, paired)
                    for i in range(4):
                        t0 = ptmp4.tile([P, 4, N], F32, tag="tmp4")
                        SQ(pair(t0, 0, 1), pair(b_t, IDX[(i, 0)], IDX[(i, 1)]))
                        SQ(pair(t0, 2, 3), pair(b_t, IDX[(i, 2)], IDX[(i, 3)]))
                        TT(pl(t0, 0), pl(t0, 0), pl(t0, 1), OP.add)
                        TT(pl(t0, 2), pl(t0, 2), pl(t0, 3), OP.add)
                        TT(pl(bn_t, IDX[(i, i)]), pl(t0, 0), pl(t0, 2), OP.add)
                    # off-diagonal: pair-products, then adds stacked per group
                    for grp in ODG:
                        ng = len(grp)
                        tg = ptmpg.tile([P, 3, 4, N], F32, tag="tmpg")
                        for m, (i, j) in enumerate(grp):
                            TT(_ap(tg, (m * 4) * N, [[N, 2], [J, T], [1, J]]),
                               pair(b_t, IDX[(i, 0)], IDX[(i, 1)]),
                               pair(b_t, IDX[(0, j)], IDX[(1, j)]), OP.mult)
                            TT(_ap(tg, (m * 4 + 2) * N, [[N, 2], [J, T], [1, J]]),
                               pair(b_t, IDX[(i, 2)], IDX[(i, 3)]),
                               pair(b_t, IDX[(2, j)], IDX[(3, j)]), OP.mult)
                        # stacked adds over the group
                        def gview(off):
                            return _ap(tg, off * N, [[4 * N, ng], [J, T], [1, J]])
                        TT(gview(0), gview(0), gview(1), OP.add)
                        TT(gview(2), gview(2), gview(3), OP.add)
                        e0 = IDX[grp[0]]
                        TT(_ap(bn_t, e0 * N, [[N, ng], [J, T], [1, J]]),
                           gview(0), gview(2), OP.add)
                    b_t = bn_t
                    if (it + 1) % norm_every == 0 and it != k_squarings - 1:
                        normalize(b_t)

            # ------- phase 4: rank-2 reduction + 2x2 squarings + output -------
            # After k0 full squarings, directions v3/v4 are suppressed to
            # (l1/l3)^(2^k0) <= 1e-7 (worst l1/l3 = 0.883 on real data), so
            # the remaining v1-vs-v2 convergence continues in the projected
            # 2x2 iteration: pick two bracket-winner columns, orthonormalize
            # (Gram-Schmidt twice: normalized cancellation noise is NOT
            # orthogonal), T = P^T B P, square T m times, expand.
            with (
                tc.tile_pool(name="w1", bufs=1) as pw1,
                tc.tile_pool(name="wfp", bufs=1) as pwf,
                tc.tile_pool(name="msk", bufs=1) as pmsk,
                tc.tile_pool(name="oi", bufs=1) as poi,
            ):
                w1 = pw1.tile([P, 12, N], F32, tag="w1")
                wf = pwf.tile([P, 8, N], F32, tag="wf")
                scr_s = ptmp.tile([P, 2, N], F32, tag="tmp2")
                msk = pmsk.tile([P, 3, N], mybir.dt.uint8, tag="msk")

                def mpl(k):
                    return _ap(msk, k * N, [[J, T], [1, J]])

                def sel(out, cmpp, on_true, on_false):
                    nc.scalar.copy(out, on_false)
                    nc.vector.copy_predicated(out, cmpp, on_true)

                def round1(base, mki, c0, c1):
                    cmpp = mpl(mki)
                    TT(cmpp, pl(b_t, IDX[(c0, c0)]), pl(b_t, IDX[(c1, c1)]), OP.is_ge)
                    for r in range(4):
                        sel(pl(w1, base + r), cmpp,
                            pl(b_t, IDX[(r, c0)]), pl(b_t, IDX[(r, c1)]))
                    TT(pl(w1, base + 4), pl(b_t, IDX[(c0, c0)]),
                       pl(b_t, IDX[(c1, c1)]), OP.max)

                round1(0, 0, 0, 1)
                round1(6, 1, 2, 3)
                cmpp = mpl(2)
                TT(cmpp, pl(w1, 4), pl(w1, 10), OP.is_ge)
                for r in range(4):
                    sel(pl(wf, r), cmpp, pl(w1, r), pl(w1, 6 + r))      # c1
                    sel(pl(wf, 4 + r), cmpp, pl(w1, 6 + r), pl(w1, r))  # c2

                sA = pl(scr_s, 0)
                sB = pl(scr_s, 1)
                TS = nc.vector.tensor_scalar

                def dot4(dst, t0, b0, t1, b1, scratch):
                    # dst = sum_r t0[b0+r]*t1[b1+r] using a [2N] pair temp
                    TT(pair(scratch, 0, 1), pair(t0, b0, b0 + 1),
                       pair(t1, b1, b1 + 1), OP.mult)
                    TT(dst, pl(scratch, 0), pl(scratch, 1), OP.add)
                    TT(pair(scratch, 0, 1), pair(t0, b0 + 2, b0 + 3),
                       pair(t1, b1 + 2, b1 + 3), OP.mult)
                    TT(pl(scratch, 0), pl(scratch, 0), pl(scratch, 1), OP.add)
                    TT(dst, dst, pl(scratch, 0), OP.add)

                def stack4(t, b):
                    return _ap(t, b * N, [[N, 4], [J, T], [1, J]])

                def bcast4(plane_ap):
                    return bass.AP(tensor=plane_ap.tensor, offset=plane_ap.offset,
                                   ap=[list(plane_ap.ap[0]), [0, 4]]
                                      + [list(d) for d in plane_ap.ap[1:]])

                def normalize_vec(t, b, scr2):
                    # t[b..b+3] /= ||.||.  Gram-Schmidt needs |u|=1 to ~1e-7
                    # (ACT sqrt table is coarse), so polish rsqrt with 2 NR
                    # iterations: r <- r*(1.5 - 0.5*x*r^2).
                    dot4(sA, t, b, t, b, scr2)
                    TS(sA, sA, 1e-30, None, OP.add)
                    nc.scalar.sqrt(sB, sA)
                    nc.vector.reciprocal_approx_fast(out=sB, in_=sB)
                    for _nr in range(2):
                        SQ(pl(scr2, 0), sB)
                        TT(pl(scr2, 0), pl(scr2, 0), sA, OP.mult)
                        TS(pl(scr2, 0), pl(scr2, 0), -0.5, 1.5, OP.mult, OP.add)
                        TT(sB, sB, pl(scr2, 0), OP.mult)
                    TT(stack4(t, b), stack4(t, b), bcast4(sB), OP.mult)

                def ortho_step(scr2):
                    # wf[4..7] -= (u1 . wf[4..7]) * u1
                    dot4(sB, wf, 0, wf, 4, scr2)
                    TT(stack4(w1, 0), stack4(wf, 0), bcast4(sB), OP.mult)
                    TT(stack4(wf, 4), stack4(wf, 4), stack4(w1, 0), OP.subtract)

                scr2 = ptmp.tile([P, 2, N], F32, tag="tmp2")
                normalize_vec(wf, 0, scr2)   # u1
                ortho_step(scr2)
                normalize_vec(wf, 4, scr2)
                ortho_step(scr2)             # re-orthogonalize (fp32 noise)
                normalize_vec(wf, 4, scr2)   # u2

                # Bu1 -> w1[0..3], Bu2 -> w1[4..7]
                for (dst, ub) in ((0, 0), (4, 4)):
                    for i in range(4):
                        TT(pair(w1, 8, 9), pair(b_t, IDX[(i, 0)], IDX[(i, 1)]),
                           pair(wf, ub, ub + 1), OP.mult)
                        TT(pair(w1, 10, 11), pair(b_t, IDX[(i, 2)], IDX[(i, 3)]),
                           pair(wf, ub + 2, ub + 3), OP.mult)
                        TT(pl(w1, 8), pl(w1, 8), pl(w1, 9), OP.add)
                        TT(pl(w1, 10), pl(w1, 10), pl(w1, 11), OP.add)
                        TT(pl(w1, dst + i), pl(w1, 8), pl(w1, 10), OP.add)

                # T (symmetric): a=T00 b=T01 d=T11 in w1[8,9,10]
                dot4(pl(w1, 8), wf, 0, w1, 0, scr2)
                dot4(pl(w1, 9), wf, 0, w1, 4, scr2)
                dot4(pl(w1, 10), wf, 4, w1, 4, scr2)

                # closed-form dominant eigenvector of T = [[a,b],[b,d]]:
                # z = (h + r, 2b) for a>=d else (2b, r - h), h = a-d,
                # r = sqrt(h^2 + (2b)^2)  (cancellation-free branch per sign;
                # exact up to fp32, so no iterated T-squarings needed)
                a_p, b_p, d_p = pl(w1, 8), pl(w1, 9), pl(w1, 10)
                h_p = pl(w1, 11)
                TT(h_p, a_p, d_p, OP.subtract)
                TT(pl(w1, 0), b_p, b_p, OP.add)            # tb = 2b
                SQ(pair(scr_s, 0, 1), pair(w1, 11, 0))     # h^2, tb^2
                TT(sA, sA, sB, OP.add)                     # s = h^2 + tb^2
                TS(sA, sA, 1e-38, None, OP.add)
                nc.scalar.sqrt(sB, sA)
                nc.vector.reciprocal_approx_fast(out=pl(w1, 1), in_=sB)
                for _nr in range(2):                       # polish rsqrt(s)
                    SQ(pl(w1, 2), pl(w1, 1))
                    TT(pl(w1, 2), pl(w1, 2), sA, OP.mult)
                    TS(pl(w1, 2), pl(w1, 2), -0.5, 1.5, OP.mult, OP.add)
                    TT(pl(w1, 1), pl(w1, 1), pl(w1, 2), OP.mult)
                TT(sB, sA, pl(w1, 1), OP.mult)             # r = s * rsqrt(s)
                TT(pl(w1, 2), h_p, sB, OP.add)             # h + r
                TT(pl(w1, 1), sB, h_p, OP.subtract)        # r - h
                cmpp = mpl(0)
                TT(cmpp, a_p, d_p, OP.is_ge)
                sel(pl(w1, 3), cmpp, pl(w1, 2), pl(w1, 0))  # z1
                sel(pl(w1, 4), cmpp, pl(w1, 0), pl(w1, 1))  # z2

                # v = z1*u1 + z2*u2 -> w1[5..8]
                TT(stack4(w1, 5), stack4(wf, 0), bcast4(pl(w1, 3)), OP.mult)
                TT(stack4(wf, 0), stack4(wf, 4), bcast4(pl(w1, 4)), OP.mult)
                TT(stack4(w1, 5), stack4(w1, 5), stack4(wf, 0), OP.add)

                rec = pl(w1, 9)
                nc.vector.reciprocal_approx_accurate(out=rec, in_=pl(w1, 8),
                                                     scratch=pl(w1, 10))
                # interleave out-muls and out-DMA by halves
                oi_t = poi.tile([P, T, 51], F32, tag="oi")
                half = max(dma_group, T // 2)
                for h0 in range(0, T, half):
                    hh = min(half, T - h0)
                    for d in range(3):
                        TT(_ap(oi_t, h0 * 51 + d, [[51, hh], [3, J]]),
                           _ap(w1, (5 + d) * N + h0 * J, [[J, hh], [1, J]]),
                           _ap(w1, 9 * N + h0 * J, [[J, hh], [1, J]]), OP.mult)
                    for g0 in range(h0, h0 + hh, dma_group):
                        g = min(dma_group, h0 + hh - g0)
                        qs[qi % 2].dma_start(
                            out=_dram_ap(out_d, g0 * P * 51,
                                         [[51, P], [51 * P, g], [1, 51]]),
                            in_=oi_t[:, g0:g0 + g, :])
                        qi += 1

            bb_st.close()
            persist.close()

        for _rep in range(repeat):
            one_pass()

    nc.compile()
    return nc


_RUNNER = None


def _get_runner():
    global _RUNNER
    if _RUNNER is None:
        import jax
        from jax.experimental.shard_map import shard_map
        from jax.sharding import Mesh, PartitionSpec

        from concourse import bass2jax

        nc = build_nc()
        bass2jax.install_neuronx_cc_hook()

        partition_name = (nc.partition_id_tensor.name
                          if nc.partition_id_tensor else None)
        in_names, out_names, out_avals, out_shapes = [], [], [], []
        for alloc in nc.m.functions[0].allocations:
            if not isinstance(alloc, mybir.MemoryLocationSet):
                continue
            name = alloc.memorylocations[0].name
            if alloc.kind == "ExternalInput":
                if name != partition_name:
                    in_names.append(name)
            elif alloc.kind == "ExternalOutput":
                out_names.append(name)
                shape = tuple(alloc.tensor_shape)
                dt = mybir.dt.np(alloc.dtype)
                out_avals.append(jax.core.ShapedArray(shape, dt))
                out_shapes.append((shape, dt))
        n_params = len(in_names)
        all_names = in_names + out_names
        if partition_name is not None:
            all_names = all_names + [partition_name]
        all_names = tuple(all_names)
        donate = tuple(range(n_params, n_params + len(out_names)))

        def _body(*args):
            operands = list(args)
            if partition_name is not None:
                operands.append(bass2jax.partition_id_tensor())
            outs = bass2jax._bass_exec_p.bind(
                *operands,
                out_avals=tuple(out_avals),
                in_names=all_names,
                out_names=tuple(out_names),
                lowering_input_output_aliases=(),
                sim_require_finite=True,
                sim_require_nnan=True,
                nc=nc,
            )
            return tuple(outs)

        import jax.numpy as jnp

        devices = jax.devices()[:NCORES]
        mesh = Mesh(np.asarray(devices), ("core",))
        nio = n_params + len(out_names)
        sharded = jax.jit(
            shard_map(_body, mesh=mesh,
                      in_specs=(PartitionSpec("core"),) * nio,
                      out_specs=(PartitionSpec("core"),) * len(out_names),
                      check_rep=False),
            donate_argnums=donate, keep_unused=True)
        in_sharding = jax.sharding.NamedSharding(mesh, PartitionSpec("core"))
        # device-side zero maker: donated output buffers without host upload
        zmaker = jax.jit(
            lambda: tuple(jnp.zeros((NCORES * s[0], *s[1:]), d)
                          for (s, d) in out_shapes),
            out_shardings=(in_sharding,) * len(out_shapes))
        _RUNNER = (sharded, in_names, out_names, out_shapes, in_sharding, zmaker)
    return _RUNNER


def kernel(points, confidences, proj_matricies, batch=None, **_unused):
    sharded, in_names, out_names, out_shapes, in_sharding, zmaker = _get_runner()

    arrs = {
        "points": np.ascontiguousarray(np.asarray(points, dtype=np.float32)),
        "confidences": np.ascontiguousarray(np.asarray(confidences, np.float32)),
        "proj_matricies": np.ascontiguousarray(np.asarray(proj_matricies, np.float32)),
    }
    import jax
    ins = [jax.device_put(arrs[n], in_sharding) for n in in_names]
    out_arrs = sharded(*ins, *zmaker())
    out = np.asarray(out_arrs[out_names.index("out")])
    return out.astype(np.float32)

